# revision 33
# baseline (speedup 1.0000x reference)
"""Trainium2 Bass kernel for nn_Attention (sparse_attention variant).

Computes, for inputs hidden/encoder_outputs [B,S,D], c_t [B,D], W [OUT,3D],
b [OUT], v [OUT]:

    cat       = concat([hidden, broadcast(c_t), encoder_outputs], axis=2)
    energy    = relu(cat @ W.T + b)            # [B, S, OUT]
    attention = energy @ v                     # [B, S]
    out       = softmax(attention, axis=1)

Strategy (8 NeuronCores, data-parallel over batch, 2 batches/core):
  - Weights are layout-prepped on host (transposed + fp16 cast, as for a
    deployed model): wT [128,16,1024] holds W1^T|W3^T chunks, w2T the W2^T
    chunks.
  - X = [hidden | enc] flows through 8 independent 512-row granules:
    SWDGE DRAM->DRAM fp32->fp16 casts write a combined [512, 2048] h|e
    scratch per granule; one xbar-transposed load per granule brings it
    into SBUF as [128, 16, 512] (chunk j<8 = hidden, j>=8 = enc, matching
    wT's layout).  Every granule owns its SBUF buffer -- no ring reuse, no
    WAR hazards, and only ~12 DMA instructions in the whole X pipeline so
    the 8-semaphore DMA windows never throttle prefetch.
  - Main loop per 128-row s-tile: accumulate pre[s,o] over 16 f-chunks x
    2 PSUM banks; VectorE adds the per-batch c2 row (c_t@W2.T + b,
    computed on PE after the first s-tile) and does fused relu*v +
    row-sum into the attention logits.
  - Softmax over S=2048 per batch: DVE free-dim reduce + GpSimd partition
    all-reduce, ScalarE exp, DVE normalize, store on the gpsimd ring.
"""

import sys
import numpy as np

for _p in ("/opt/trn_rl_repo",):
    if _p not in sys.path:
        sys.path.insert(0, _p)

import concourse.bass as bass
import concourse.bacc as bacc
import concourse.tile as tile
from concourse import mybir, bass_isa
from concourse.bass_utils import run_bass_kernel_spmd

F32 = mybir.dt.float32
F16 = mybir.dt.float16
BF16 = mybir.dt.bfloat16
AF = mybir.ActivationFunctionType
ALU = mybir.AluOpType

B, S, D, OUT = 16, 2048, 1024, 1024
N_CORES = 8
B_LOC = B // N_CORES            # batches per core
S_LOC = B_LOC * S               # 4096 rows of X per core
N_ST = S_LOC // 128             # 32 s-tiles per core
ST_PER_B = S // 128             # 16 s-tiles per batch
FC = D // 128                   # 8 feature chunks per tensor
NB = OUT // 512                 # 2 PSUM banks across OUT
G_ROWS = 512                    # rows per X granule
NG = S_LOC // G_ROWS            # 8 granules
ST_PER_G = G_ROWS // 128        # 4 s-tiles per granule


def build_nc():
    nc = bacc.Bacc("TRN2", target_bir_lowering=False, debug=False,
                   num_devices=N_CORES, dynamic_dma_scratch_size=32768)

    hid = nc.dram_tensor("hidden", [S_LOC, D], F32, kind="ExternalInput").ap()
    enc = nc.dram_tensor("enc", [S_LOC, D], F32, kind="ExternalInput").ap()
    ct = nc.dram_tensor("ct", [B_LOC, D], F32, kind="ExternalInput").ap()
    wTd = nc.dram_tensor("wT", [128, 2 * FC, OUT], F16,
                         kind="ExternalInput").ap()
    w2Td = nc.dram_tensor("w2T", [128, FC, OUT], F16,
                          kind="ExternalInput").ap()
    bd = nc.dram_tensor("b", [OUT], F16, kind="ExternalInput").ap()
    vd = nc.dram_tensor("v", [OUT], F16, kind="ExternalInput").ap()
    outd = nc.dram_tensor("out", [B_LOC, 128, ST_PER_B], F32,
                          kind="ExternalOutput").ap()
    scr_h = [nc.dram_tensor(f"scr_h{g}", [G_ROWS, D], F16).ap()
             for g in range(NG)]
    scr_e = [nc.dram_tensor(f"scr_e{g}", [G_ROWS, D], F16).ap()
             for g in range(NG)]

    with tile.TileContext(nc) as tc:
        with (
            tc.tile_pool(name="const", bufs=1) as cpool,
            tc.tile_pool(name="wT", bufs=1) as wpool,
            tc.tile_pool(name="xT", bufs=1) as xTp,
            tc.tile_pool(name="sm", bufs=2) as smpool,
            tc.tile_pool(name="eps", bufs=3, space=bass.MemorySpace.PSUM) as eps,
            tc.tile_pool(name="pps", bufs=1, space=bass.MemorySpace.PSUM) as pps,
        ):
            ones_k1 = cpool.tile([1, 128], F16)
            nc.vector.memset(ones_k1[:], 1.0)
            att_all = cpool.tile([128, N_ST], F32)   # attention logits

            # ---- X granules: SWDGE casts (contiguous scratch, ~32 descs
            # each, so the dynamic-DMA scratch never throttles) then two
            # transposed loads per granule into its own SBUF buffer --------
            for g in range(NG):
                rows = slice(g * G_ROWS, (g + 1) * G_ROWS)
                nc.gpsimd.dma_start(scr_h[g][:], hid[rows, :])
                nc.gpsimd.dma_start(scr_e[g][:], enc[rows, :])

            # small constants after the casts on the gpsimd ring
            ctT_f = cpool.tile([128, FC, B_LOC], F32)
            for bb in range(B_LOC):
                nc.gpsimd.dma_start(ctT_f[:, :, bb],
                                    ct[bb].rearrange("(fc p) -> p fc", p=128))
            b_h = cpool.tile([1, OUT], F16)
            nc.gpsimd.dma_start(b_h[:], bd[None, :])
            v_h = cpool.tile([1, OUT], F16)
            nc.gpsimd.dma_start(v_h[:], vd[None, :])
            ctT_h = cpool.tile([128, FC, B_LOC], F16)
            nc.vector.tensor_copy(ctT_h[:], ctT_f[:])

            wT = wpool.tile([128, 2 * FC, OUT], F16)
            nc.sync.dma_start(wT[:, :FC, :], wTd[:, :FC, :])
            nc.scalar.dma_start(wT[:, FC:, :], wTd[:, FC:, :])

            xts = []
            w2T = None
            for g in range(NG):
                xt = xTp.tile([128, 2 * FC, G_ROWS], F16, tag=f"xt{g}")
                nc.scalar.dma_start(xt[:, :FC, :], scr_h[g][:], transpose=True)
                nc.scalar.dma_start(xt[:, FC:, :], scr_e[g][:], transpose=True)
                xts.append(xt)
                if g == 0:
                    w2T = wpool.tile([128, FC, OUT], F16, tag="w2T")
                    nc.scalar.dma_start(w2T[:], w2Td)

            c2bc_sb = []

            def emit_c2_vbc():
                # c2[b,:] = c_t[b] @ W2.T + b, broadcast to 128 rows
                for bb in range(B_LOC):
                    c2_ps = pps.tile([1, OUT], F32, tag="pp")
                    for ob in range(NB):
                        sl = slice(ob * 512, (ob + 1) * 512)
                        for fc in range(FC):
                            nc.tensor.matmul(c2_ps[:, sl],
                                             ctT_h[:, fc, bb:bb + 1],
                                             w2T[:, fc, sl],
                                             start=(fc == 0), stop=False)
                        nc.tensor.matmul(c2_ps[:, sl], ones_k1[:, :1],
                                         b_h[:, sl], start=False, stop=True)
                    c2b = cpool.tile([1, OUT], F16, tag=f"c2_{bb}")
                    nc.vector.tensor_copy(c2b[:], c2_ps[:])
                    c2bc_ps = pps.tile([128, OUT], F32, tag="pp")
                    for ob in range(NB):
                        sl = slice(ob * 512, (ob + 1) * 512)
                        nc.tensor.matmul(c2bc_ps[:, sl], ones_k1[:],
                                         c2b[:, sl], start=True, stop=True)
                    c2bc = cpool.tile([128, OUT], F16, tag=f"c2bc_{bb}")
                    nc.vector.tensor_copy(c2bc[:], c2bc_ps[:])
                    c2bc_sb.append(c2bc)
                # vbc[p, o] = v[o] (fp16) for the fused relu*v epilogue
                vbc_ps = pps.tile([128, OUT], F32, tag="pp")
                for ob in range(NB):
                    sl = slice(ob * 512, (ob + 1) * 512)
                    nc.tensor.matmul(vbc_ps[:, sl], ones_k1[:], v_h[:, sl],
                                     start=True, stop=True)
                vbc = cpool.tile([128, OUT], F16)
                nc.vector.tensor_copy(vbc[:], vbc_ps[:])
                return vbc

            def emit_softmax(bb):
                sl = slice(bb * ST_PER_B, (bb + 1) * ST_PER_B)
                m1 = smpool.tile([128, 1], F32, tag="m1")
                nc.vector.tensor_reduce(m1[:], att_all[:, sl],
                                        axis=mybir.AxisListType.X,
                                        op=ALU.max)
                mall = smpool.tile([128, 1], F32, tag="mall")
                nc.gpsimd.partition_all_reduce(mall[:], m1[:], channels=128,
                                               reduce_op=bass_isa.ReduceOp.max)
                nmall = smpool.tile([128, 1], F32, tag="nmall")
                nc.vector.tensor_scalar_mul(nmall[:], mall[:], -1.0)
                ex = smpool.tile([128, ST_PER_B], F32, tag="ex")
                rs = smpool.tile([128, 1], F32, tag="rs")
                nc.scalar.activation(ex[:], att_all[:, sl], AF.Exp,
                                     bias=nmall[:], accum_out=rs[:])
                tot = smpool.tile([128, 1], F32, tag="tot")
                nc.gpsimd.partition_all_reduce(tot[:], rs[:], channels=128,
                                               reduce_op=bass_isa.ReduceOp.add)
                rec = smpool.tile([128, 1], F32, tag="rec")
                nc.vector.reciprocal(rec[:], tot[:])
                res_t = smpool.tile([128, ST_PER_B], F32, tag="res")
                nc.vector.tensor_scalar_mul(res_t[:], ex[:], rec[:])
                nc.gpsimd.dma_start(outd[bb], res_t[:])

            # ---- main loop ------------------------------------------------
            vbc = None
            for st in range(N_ST):
                b_idx = st // ST_PER_B
                xt = xts[st // ST_PER_G]
                ssl = slice((st % ST_PER_G) * 128, (st % ST_PER_G) * 128 + 128)

                e_ps = eps.tile([128, OUT], F32, tag="eps")
                for ob in range(NB):
                    sl = slice(ob * 512, (ob + 1) * 512)
                    for j in range(2 * FC):
                        nc.tensor.matmul(e_ps[:, sl], xt[:, j, ssl],
                                         wT[:, j, sl],
                                         start=(j == 0), stop=(j == 2 * FC - 1))

                if st == 0:
                    vbc = emit_c2_vbc()

                # pre += c2[b] (broadcast), then
                # att[st] = sum_o relu(pre) * v  (fused on VectorE).
                # relu_out is a dummy destination (only accum_out is used);
                # it aliases w2T's buffer, which is dead after c2.
                nc.vector.tensor_add(e_ps[:], e_ps[:], c2bc_sb[b_idx][:])
                relu_out = wpool.tile([128, OUT], BF16, tag="w2T")
                nc.vector.scalar_tensor_tensor(
                    relu_out[:], e_ps[:], 0.0, vbc[:],
                    op0=ALU.max, op1=ALU.mult,
                    accum_out=att_all[:, st:st + 1])
                if st % ST_PER_B == ST_PER_B - 1:
                    emit_softmax(st // ST_PER_B)

    nc.compile()
    return nc


_NC = None


def _get_nc():
    global _NC
    if _NC is None:
        _NC = build_nc()
    return _NC


def _prep_weights(W, b, v):
    W = np.ascontiguousarray(W, dtype=np.float32)
    # wT[p, j, o] = W1[o, j*128+p] for j<8, W3[o, (j-8)*128+p] for j>=8
    W13T = np.concatenate([W[:, :D].T, W[:, 2 * D:].T], axis=0)  # [2D, OUT]
    wT = np.ascontiguousarray(
        W13T.reshape(2 * FC, 128, OUT).transpose(1, 0, 2).astype(np.float16))
    w2T = np.ascontiguousarray(
        W[:, D:2 * D].T.reshape(FC, 128, OUT).transpose(1, 0, 2)
        .astype(np.float16))
    b = np.ascontiguousarray(b, dtype=np.float16)
    v = np.ascontiguousarray(v, dtype=np.float16)
    return wT, w2T, b, v


def _in_maps(hidden, encoder_outputs, c_t, W, b, v):
    hidden = np.ascontiguousarray(hidden, dtype=np.float32)
    encoder_outputs = np.ascontiguousarray(encoder_outputs, dtype=np.float32)
    c_t = np.ascontiguousarray(c_t, dtype=np.float32)
    wT, w2T, b, v = _prep_weights(W, b, v)
    maps = []
    for i in range(N_CORES):
        bs = slice(i * B_LOC, (i + 1) * B_LOC)
        maps.append({
            "hidden": hidden[bs].reshape(S_LOC, D),
            "enc": encoder_outputs[bs].reshape(S_LOC, D),
            "ct": c_t[bs],
            "wT": wT, "w2T": w2T, "b": b, "v": v,
        })
    return maps


def run(hidden, encoder_outputs, c_t, W, b, v, trace=False, tmpdir=None):
    nc = _get_nc()
    maps = _in_maps(hidden, encoder_outputs, c_t, W, b, v)
    res = run_bass_kernel_spmd(nc, maps, list(range(N_CORES)), trace=trace,
                               tmpdir=tmpdir)
    # device out layout is [B_LOC, 128, ST_PER_B] with s = stl*128 + p
    out = np.concatenate(
        [res.results[i]["out"].transpose(0, 2, 1).reshape(B_LOC, S)
         for i in range(N_CORES)], axis=0)
    return out, res


def kernel(hidden, encoder_outputs, c_t, W, b, v):
    out, _ = run(hidden, encoder_outputs, c_t, W, b, v)
    return out


# revision 43
# speedup vs baseline: 1.2796x; 1.2796x over previous
"""Trainium2 Bass kernel for nn_Attention (sparse_attention variant).

Computes, for inputs hidden/encoder_outputs [B,S,D], c_t [B,D], W [OUT,3D],
b [OUT], v [OUT]:

    cat       = concat([hidden, broadcast(c_t), encoder_outputs], axis=2)
    energy    = relu(cat @ W.T + b)            # [B, S, OUT]
    attention = energy @ v                     # [B, S]
    out       = softmax(attention, axis=1)

Strategy (8 NeuronCores, data-parallel over batch, 2 batches/core):
  - Weights are layout-prepped on host (transposed + fp16 cast, as for a
    deployed model): wT [128,16,1024] holds W1^T|W3^T chunks, w2T the W2^T
    chunks.
  - X = [hidden | enc] flows through 8 independent 512-row granules:
    SWDGE DRAM->DRAM fp32->fp16 casts write a combined [512, 2048] h|e
    scratch per granule; one xbar-transposed load per granule brings it
    into SBUF as [128, 16, 512] (chunk j<8 = hidden, j>=8 = enc, matching
    wT's layout).  Every granule owns its SBUF buffer -- no ring reuse, no
    WAR hazards, and only ~12 DMA instructions in the whole X pipeline so
    the 8-semaphore DMA windows never throttle prefetch.
  - Main loop per 128-row s-tile: accumulate pre[s,o] over 16 f-chunks x
    2 PSUM banks; VectorE adds the per-batch c2 row (c_t@W2.T + b,
    computed on PE after the first s-tile) and does fused relu*v +
    row-sum into the attention logits.
  - Softmax over S=2048 per batch: DVE free-dim reduce + GpSimd partition
    all-reduce, ScalarE exp, DVE normalize, store on the gpsimd ring.
"""

import sys
import numpy as np

for _p in ("/opt/trn_rl_repo",):
    if _p not in sys.path:
        sys.path.insert(0, _p)

import concourse.bass as bass
import concourse.bacc as bacc
import concourse.tile as tile
from concourse import mybir, bass_isa
from concourse.bass_utils import run_bass_kernel_spmd

F32 = mybir.dt.float32
F16 = mybir.dt.float16
BF16 = mybir.dt.bfloat16
AF = mybir.ActivationFunctionType
ALU = mybir.AluOpType

B, S, D, OUT = 16, 2048, 1024, 1024
N_CORES = 8
B_LOC = B // N_CORES            # batches per core
S_LOC = B_LOC * S               # 4096 rows of X per core
N_ST = S_LOC // 128             # 32 s-tiles per core
ST_PER_B = S // 128             # 16 s-tiles per batch
FC = D // 128                   # 8 feature chunks per tensor
NB = OUT // 512                 # 2 PSUM banks across OUT
G_ROWS = 512                    # rows per X granule
NG = S_LOC // G_ROWS            # 8 granules
ST_PER_G = G_ROWS // 128        # 4 s-tiles per granule


def build_nc():
    nc = bacc.Bacc("TRN2", target_bir_lowering=False, debug=False,
                   num_devices=N_CORES, dynamic_dma_scratch_size=32768)

    hid = nc.dram_tensor("hidden", [S_LOC, D], F32, kind="ExternalInput").ap()
    enc = nc.dram_tensor("enc", [S_LOC, D], F32, kind="ExternalInput").ap()
    ct = nc.dram_tensor("ct", [B_LOC, D], F32, kind="ExternalInput").ap()
    wTd = nc.dram_tensor("wT", [128, 2 * FC, OUT], F16,
                         kind="ExternalInput").ap()
    w2Td = nc.dram_tensor("w2T", [128, FC, OUT], F16,
                          kind="ExternalInput").ap()
    bd = nc.dram_tensor("b", [OUT], F16, kind="ExternalInput").ap()
    vd = nc.dram_tensor("v", [OUT], F16, kind="ExternalInput").ap()
    outd = nc.dram_tensor("out", [B_LOC, 128, ST_PER_B], F32,
                          kind="ExternalOutput").ap()
    scr_h = [nc.dram_tensor(f"scr_h{g}", [G_ROWS, D], F16).ap()
             for g in range(NG)]
    scr_e = [nc.dram_tensor(f"scr_e{g}", [G_ROWS, D], F16).ap()
             for g in range(NG)]

    with tile.TileContext(nc) as tc:
        with (
            tc.tile_pool(name="const", bufs=1) as cpool,
            tc.tile_pool(name="wT", bufs=1) as wpool,
            tc.tile_pool(name="xT", bufs=1) as xTp,
            tc.tile_pool(name="sm", bufs=2) as smpool,
            tc.tile_pool(name="eps", bufs=3, space=bass.MemorySpace.PSUM) as eps,
            tc.tile_pool(name="pps", bufs=1, space=bass.MemorySpace.PSUM) as pps,
        ):
            ones_k1 = cpool.tile([1, 128], F16)
            nc.vector.memset(ones_k1[:], 1.0)
            att_all = cpool.tile([128, N_ST], F32)   # attention logits

            # ---- X granules + weights, emitted in the exact per-granule
            # pipeline order we want executed (Tile enforces its modeled
            # serial DMA-engine order with cross-queue sems, so emission
            # order IS the execution schedule).  Casts cap descriptors at
            # 4096 elems to keep the cost model sane. ----------------------
            def emit_cast(g):
                rows = slice(g * G_ROWS, (g + 1) * G_ROWS)
                nc.gpsimd.dma_start(scr_h[g][:], hid[rows, :])
                nc.gpsimd.dma_start(scr_e[g][:], enc[rows, :])

            # xh_t[st]/xe_t[st]: the transposed tiles covering s-tile st,
            # with the slice of the tile that holds it.
            xh_t = [None] * N_ST
            xe_t = [None] * N_ST

            def emit_sub(k):
                # 128-row sub-granule k (s-tile k) for a fast pipeline head
                rows = slice(k * 128, (k + 1) * 128)
                nc.gpsimd.dma_start(scr_h[0][rows, :], hid[rows, :])
                xh = xTp.tile([128, FC, 128], F16, tag=f"xh0s{k}")
                nc.scalar.dma_start(xh[:], scr_h[0][rows, :], transpose=True)
                xh_t[k] = (xh, slice(0, 128))
                nc.gpsimd.dma_start(scr_e[0][rows, :], enc[rows, :])
                xe = xTp.tile([128, FC, 128], F16, tag=f"xe0s{k}")
                nc.scalar.dma_start(xe[:], scr_e[0][rows, :], transpose=True)
                xe_t[k] = (xe, slice(0, 128))

            def emit_granule(g):
                emit_cast(g)
                xh = xTp.tile([128, FC, G_ROWS], F16, tag=f"xh{g}")
                nc.scalar.dma_start(xh[:], scr_h[g][:], transpose=True)
                xe = xTp.tile([128, FC, G_ROWS], F16, tag=f"xe{g}")
                nc.scalar.dma_start(xe[:], scr_e[g][:], transpose=True)
                for r in range(ST_PER_G):
                    st = g * ST_PER_G + r
                    xh_t[st] = (xh, slice(r * 128, (r + 1) * 128))
                    xe_t[st] = (xe, slice(r * 128, (r + 1) * 128))

            # head: 128-row sub-granules interleaved with the weight loads
            rows0 = slice(0, 128)
            nc.gpsimd.dma_start(scr_h[0][rows0, :], hid[rows0, :])
            wTlo = wpool.tile([128, FC, OUT], F16, tag="wTlo")
            nc.sync.dma_start(wTlo[:], wTd[:, :FC, :])
            xh00 = xTp.tile([128, FC, 128], F16, tag="xh0s0")
            nc.scalar.dma_start(xh00[:], scr_h[0][rows0, :], transpose=True)
            xh_t[0] = (xh00, slice(0, 128))
            nc.gpsimd.dma_start(scr_e[0][rows0, :], enc[rows0, :])
            wThi = wpool.tile([128, FC, OUT], F16, tag="wThi")
            nc.scalar.dma_start(wThi[:], wTd[:, FC:, :])
            xe00 = xTp.tile([128, FC, 128], F16, tag="xe0s0")
            nc.scalar.dma_start(xe00[:], scr_e[0][rows0, :], transpose=True)
            xe_t[0] = (xe00, slice(0, 128))
            for k in range(1, ST_PER_G):
                emit_sub(k)

            w2T = wpool.tile([128, FC, OUT], F16, tag="w2T")
            nc.scalar.dma_start(w2T[:], w2Td)

            # small constants on the gpsimd ring
            ctT_f = cpool.tile([128, FC, B_LOC], F32)
            for bb in range(B_LOC):
                nc.gpsimd.dma_start(ctT_f[:, :, bb],
                                    ct[bb].rearrange("(fc p) -> p fc", p=128))
            b_h = cpool.tile([1, OUT], F16)
            nc.gpsimd.dma_start(b_h[:], bd[None, :])
            v_h = cpool.tile([1, OUT], F16)
            nc.gpsimd.dma_start(v_h[:], vd[None, :])
            ctT_h = cpool.tile([128, FC, B_LOC], F16)
            nc.vector.tensor_copy(ctT_h[:], ctT_f[:])

            for g in range(1, NG):
                emit_granule(g)

            c2bc_sb = []

            def emit_c2_vbc():
                # c2[b,:] = c_t[b] @ W2.T + b, broadcast to 128 rows
                for bb in range(B_LOC):
                    c2_ps = pps.tile([1, OUT], F32, tag="pp")
                    for ob in range(NB):
                        sl = slice(ob * 512, (ob + 1) * 512)
                        for fc in range(FC):
                            nc.tensor.matmul(c2_ps[:, sl],
                                             ctT_h[:, fc, bb:bb + 1],
                                             w2T[:, fc, sl],
                                             start=(fc == 0), stop=False)
                        nc.tensor.matmul(c2_ps[:, sl], ones_k1[:, :1],
                                         b_h[:, sl], start=False, stop=True)
                    c2b = cpool.tile([1, OUT], F16, tag=f"c2_{bb}")
                    nc.vector.tensor_copy(c2b[:], c2_ps[:])
                    c2bc_ps = pps.tile([128, OUT], F32, tag="pp")
                    for ob in range(NB):
                        sl = slice(ob * 512, (ob + 1) * 512)
                        nc.tensor.matmul(c2bc_ps[:, sl], ones_k1[:],
                                         c2b[:, sl], start=True, stop=True)
                    c2bc = cpool.tile([128, OUT], F16, tag=f"c2bc_{bb}")
                    nc.vector.tensor_copy(c2bc[:], c2bc_ps[:])
                    c2bc_sb.append(c2bc)
                # vbc[p, o] = v[o] (fp16) for the fused relu*v epilogue
                vbc_ps = pps.tile([128, OUT], F32, tag="pp")
                for ob in range(NB):
                    sl = slice(ob * 512, (ob + 1) * 512)
                    nc.tensor.matmul(vbc_ps[:, sl], ones_k1[:], v_h[:, sl],
                                     start=True, stop=True)
                vbc = cpool.tile([128, OUT], F16)
                nc.vector.tensor_copy(vbc[:], vbc_ps[:])
                return vbc

            def emit_softmax(bb):
                sl = slice(bb * ST_PER_B, (bb + 1) * ST_PER_B)
                m1 = smpool.tile([128, 1], F32, tag="m1")
                nc.vector.tensor_reduce(m1[:], att_all[:, sl],
                                        axis=mybir.AxisListType.X,
                                        op=ALU.max)
                mall = smpool.tile([128, 1], F32, tag="mall")
                nc.gpsimd.partition_all_reduce(mall[:], m1[:], channels=128,
                                               reduce_op=bass_isa.ReduceOp.max)
                nmall = smpool.tile([128, 1], F32, tag="nmall")
                nc.vector.tensor_scalar_mul(nmall[:], mall[:], -1.0)
                ex = smpool.tile([128, ST_PER_B], F32, tag="ex")
                rs = smpool.tile([128, 1], F32, tag="rs")
                nc.scalar.activation(ex[:], att_all[:, sl], AF.Exp,
                                     bias=nmall[:], accum_out=rs[:])
                tot = smpool.tile([128, 1], F32, tag="tot")
                nc.gpsimd.partition_all_reduce(tot[:], rs[:], channels=128,
                                               reduce_op=bass_isa.ReduceOp.add)
                rec = smpool.tile([128, 1], F32, tag="rec")
                nc.vector.reciprocal(rec[:], tot[:])
                res_t = smpool.tile([128, ST_PER_B], F32, tag="res")
                nc.vector.tensor_scalar_mul(res_t[:], ex[:], rec[:])
                nc.gpsimd.dma_start(outd[bb], res_t[:])

            # ---- main loop ------------------------------------------------
            def emit_epilogue(st, e_ps):
                # pre += c2[b] (broadcast), then
                # att[st] = sum_o relu(pre) * v  (fused on VectorE).
                # relu_out is a dummy destination (only accum_out is used);
                # it aliases w2T's buffer, which is dead after c2.
                nc.vector.tensor_add(e_ps[:], e_ps[:],
                                     c2bc_sb[st // ST_PER_B][:])
                nc.vector.scalar_tensor_tensor(
                    e_ps[:], e_ps[:], 0.0, vbc[:],
                    op0=ALU.max, op1=ALU.mult,
                    accum_out=att_all[:, st:st + 1])
                if st % ST_PER_B == ST_PER_B - 1:
                    emit_softmax(st // ST_PER_B)

            vbc = None
            pend = []
            for st in range(N_ST):
                xh, hsl = xh_t[st]
                xe, esl = xe_t[st]

                e_ps = eps.tile([128, OUT], F32, tag="eps")
                for ob in range(NB):
                    sl = slice(ob * 512, (ob + 1) * 512)
                    for fc in range(FC):
                        nc.tensor.matmul(e_ps[:, sl], xh[:, fc, hsl],
                                         wTlo[:, fc, sl],
                                         start=(fc == 0), stop=False)
                    for fc in range(FC):
                        nc.tensor.matmul(e_ps[:, sl], xe[:, fc, esl],
                                         wThi[:, fc, sl],
                                         start=False, stop=(fc == FC - 1))

                if st == 0:
                    vbc = emit_c2_vbc()
                    for pst, peps in pend:
                        emit_epilogue(pst, peps)
                    pend = []
                if vbc is None:
                    pend.append((st, e_ps))
                else:
                    emit_epilogue(st, e_ps)

    nc.compile()
    return nc


_NC = None


def _get_nc():
    global _NC
    if _NC is None:
        _NC = build_nc()
    return _NC


def _prep_weights(W, b, v):
    W = np.ascontiguousarray(W, dtype=np.float32)
    # wT[p, j, o] = W1[o, j*128+p] for j<8, W3[o, (j-8)*128+p] for j>=8
    W13T = np.concatenate([W[:, :D].T, W[:, 2 * D:].T], axis=0)  # [2D, OUT]
    wT = np.ascontiguousarray(
        W13T.reshape(2 * FC, 128, OUT).transpose(1, 0, 2).astype(np.float16))
    w2T = np.ascontiguousarray(
        W[:, D:2 * D].T.reshape(FC, 128, OUT).transpose(1, 0, 2)
        .astype(np.float16))
    b = np.ascontiguousarray(b, dtype=np.float16)
    v = np.ascontiguousarray(v, dtype=np.float16)
    return wT, w2T, b, v


def _in_maps(hidden, encoder_outputs, c_t, W, b, v):
    hidden = np.ascontiguousarray(hidden, dtype=np.float32)
    encoder_outputs = np.ascontiguousarray(encoder_outputs, dtype=np.float32)
    c_t = np.ascontiguousarray(c_t, dtype=np.float32)
    wT, w2T, b, v = _prep_weights(W, b, v)
    maps = []
    for i in range(N_CORES):
        bs = slice(i * B_LOC, (i + 1) * B_LOC)
        maps.append({
            "hidden": hidden[bs].reshape(S_LOC, D),
            "enc": encoder_outputs[bs].reshape(S_LOC, D),
            "ct": c_t[bs],
            "wT": wT, "w2T": w2T, "b": b, "v": v,
        })
    return maps


def run(hidden, encoder_outputs, c_t, W, b, v, trace=False, tmpdir=None):
    nc = _get_nc()
    maps = _in_maps(hidden, encoder_outputs, c_t, W, b, v)
    res = run_bass_kernel_spmd(nc, maps, list(range(N_CORES)), trace=trace,
                               tmpdir=tmpdir)
    # device out layout is [B_LOC, 128, ST_PER_B] with s = stl*128 + p
    out = np.concatenate(
        [res.results[i]["out"].transpose(0, 2, 1).reshape(B_LOC, S)
         for i in range(N_CORES)], axis=0)
    return out, res


def kernel(hidden, encoder_outputs, c_t, W, b, v):
    out, _ = run(hidden, encoder_outputs, c_t, W, b, v)
    return out
